# revision 6
# baseline (speedup 1.0000x reference)
"""Log-domain Sinkhorn (B=16, N=M=2048, eps=0.05, 50 iters) on 8 trn2 cores.

Strategy: data-parallel over batch (2 batches/core, sequential per core).
Math is done in the linear domain: EK = exp(-cost/eps) stays resident in
SBUF as bf16 in both layouts (EK and EK^T); each half-iteration is a
matrix-vector product on the tensor engine (matrix stationary, vector
moving), with vectors kept partition-major [128,16] so the reciprocal /
rescale glue runs at full vector-engine width. Final transport plan is
recomputed from fresh f32 cost tiles: T = exp(-cost/eps + log_u) * ev_bcast.
"""
import sys

sys.path.insert(0, "/opt/trn_rl_repo")

import numpy as np
from contextlib import ExitStack

import concourse.bass as bass
import concourse.tile as tile
from concourse import bacc, mybir
from concourse.bass_utils import run_bass_kernel_spmd
from concourse.masks import make_identity

EPS = 0.05
ITERS = 50
N = 2048
P = 128
NCH = N // P  # 16 chunks
BPC = 2      # batches per core
NCORES = 8

F32 = mybir.dt.float32
BF16 = mybir.dt.bfloat16
AF = mybir.ActivationFunctionType


def _sinkhorn_kernel(tc, out_ap, cost_ap, src_ap, tgt_ap):
    nc = tc.nc
    with ExitStack() as ctx:
        consts = ctx.enter_context(tc.tile_pool(name="consts", bufs=1))
        ekp = ctx.enter_context(tc.tile_pool(name="ek", bufs=1))
        vec = ctx.enter_context(tc.tile_pool(name="vec", bufs=1))
        stage = ctx.enter_context(tc.tile_pool(name="stage", bufs=2))
        mid = ctx.enter_context(tc.tile_pool(name="mid", bufs=2))
        ostage = ctx.enter_context(tc.tile_pool(name="ostage", bufs=2))
        psum = ctx.enter_context(tc.tile_pool(name="psum", bufs=1, space="PSUM"))
        dram = ctx.enter_context(tc.tile_pool(name="dram", bufs=1, space="DRAM"))

        identity = consts.tile([P, P], F32)
        make_identity(nc, identity)
        ones_row = consts.tile([1, P], F32)
        nc.vector.memset(ones_row, 1.0)

        eka = ekp.tile([P, NCH, N], BF16, tag="eka")  # [i', ic, j] = EK[ic*128+i', j]
        ekb = ekp.tile([P, NCH, N], BF16, tag="ekb")  # [j', jc, i] = EK[i, jc*128+j']
        ekdram = dram.tile([N, N], BF16)

        r_lin = vec.tile([P, NCH], F32, tag="r_lin")
        c_lin = vec.tile([P, NCH], F32, tag="c_lin")
        eu_f = vec.tile([P, NCH], F32, tag="eu_f")
        ev_f = vec.tile([P, NCH], F32, tag="ev_f")
        tmp_a = vec.tile([P, NCH], F32, tag="tmp_a")
        tmp_b = vec.tile([P, NCH], F32, tag="tmp_b")
        log_u = vec.tile([P, NCH], F32, tag="log_u")
        eu_bf = vec.tile([P, NCH], BF16, tag="eu_bf")
        ev_bf = vec.tile([P, NCH], BF16, tag="ev_bf")
        evrow = vec.tile([1, N], F32, tag="evrow")
        rc_raw = vec.tile([P, NCH], F32, tag="rc_raw")
        cc_raw = vec.tile([P, NCH], F32, tag="cc_raw")

        psum_su = psum.tile([P, NCH], F32, tag="su")
        psum_sv = psum.tile([P, NCH], F32, tag="sv")
        psum_evrow = psum.tile([1, 512], F32, tag="evrow")
        psum_evb = psum.tile([P, N], F32, tag="evb")

        for b in range(BPC):
            # ---- setup: marginals, EK (both layouts) ----
            rv = src_ap[b].rearrange("(cc p) -> p cc", p=P)
            cv = tgt_ap[b].rearrange("(cc p) -> p cc", p=P)
            nc.sync.dma_start(out=rc_raw, in_=rv)
            nc.sync.dma_start(out=cc_raw, in_=cv)
            nc.vector.tensor_scalar_add(r_lin, rc_raw, 1e-12)
            nc.vector.tensor_scalar_add(c_lin, cc_raw, 1e-12)
            nc.vector.memset(ev_bf, 1.0)

            for ic in range(NCH):
                ct = stage.tile([P, N], F32)
                nc.sync.dma_start(out=ct, in_=cost_ap[b, ic * P:(ic + 1) * P, :])
                nc.scalar.activation(eka[:, ic, :], ct, AF.Exp, scale=-1.0 / EPS)
                nc.sync.dma_start(out=ekdram[ic * P:(ic + 1) * P, :], in_=eka[:, ic, :])
            for jc in range(NCH):
                nc.sync.dma_start_transpose(
                    out=ekb[:, jc, :], in_=ekdram[:, jc * P:(jc + 1) * P]
                )

            # ---- 50 Sinkhorn iterations, all on-chip ----
            with tc.For_i(0, ITERS, 1, hint_engines=(mybir.EngineType.PE,)):
                # u-update: su_i = sum_j EK[i,j] * ev_j  (contract j => EK^T layout)
                for ic in range(NCH):
                    for jc in range(NCH):
                        nc.tensor.matmul(
                            psum_su[:, ic:ic + 1],
                            ekb[:, jc, ic * P:(ic + 1) * P],
                            ev_bf[:, jc:jc + 1],
                            start=(jc == 0),
                            stop=(jc == NCH - 1),
                        )
                nc.vector.reciprocal(tmp_a, psum_su)
                nc.vector.tensor_mul(eu_f, tmp_a, r_lin)
                nc.vector.tensor_copy(eu_bf, eu_f)
                # v-update: sv_j = sum_i EK[i,j] * eu_i  (contract i => EK layout)
                for jc in range(NCH):
                    for ic in range(NCH):
                        nc.tensor.matmul(
                            psum_sv[:, jc:jc + 1],
                            eka[:, ic, jc * P:(jc + 1) * P],
                            eu_bf[:, ic:ic + 1],
                            start=(ic == 0),
                            stop=(ic == NCH - 1),
                        )
                nc.vector.reciprocal(tmp_b, psum_sv)
                nc.vector.tensor_mul(ev_f, tmp_b, c_lin)
                nc.vector.tensor_copy(ev_bf, ev_f)

            # ---- finale: T = exp(-cost/eps + log_u_i) * ev_j ----
            nc.scalar.activation(log_u, eu_f, AF.Ln)
            # broadcast ev across partitions: per-chunk PE transpose into a
            # free-major [1, 2048] row, then outer-product with ones into
            # PSUM [128, 2048]
            for q in range(4):
                for k in range(4):
                    jc = 4 * q + k
                    nc.tensor.transpose(
                        psum_evrow[:, k * P:(k + 1) * P], ev_f[:, jc:jc + 1], identity
                    )
                nc.vector.tensor_copy(evrow[:, q * 512:(q + 1) * 512], psum_evrow)
            for q in range(4):
                nc.tensor.matmul(
                    psum_evb[:, q * 512:(q + 1) * 512],
                    ones_row,
                    evrow[:, q * 512:(q + 1) * 512],
                    start=True,
                    stop=True,
                )
            for ic in range(NCH):
                ct2 = stage.tile([P, N], F32)
                nc.sync.dma_start(out=ct2, in_=cost_ap[b, ic * P:(ic + 1) * P, :])
                xt = mid.tile([P, N], F32)
                nc.scalar.activation(
                    xt, ct2, AF.Exp, bias=log_u[:, ic:ic + 1], scale=-1.0 / EPS
                )
                ot = ostage.tile([P, N], F32)
                nc.vector.tensor_mul(ot, xt, psum_evb)
                nc.sync.dma_start(out=out_ap[b, ic * P:(ic + 1) * P, :], in_=ot)


_CACHE = {}


def _get_compiled():
    if "nc" not in _CACHE:
        nc = bacc.Bacc(
            "TRN2", target_bir_lowering=False, debug=False, num_devices=NCORES
        )
        cost = nc.dram_tensor("cost", [BPC, N, N], F32, kind="ExternalInput").ap()
        src = nc.dram_tensor("src", [BPC, N], F32, kind="ExternalInput").ap()
        tgt = nc.dram_tensor("tgt", [BPC, N], F32, kind="ExternalInput").ap()
        out = nc.dram_tensor("out", [BPC, N, N], F32, kind="ExternalOutput").ap()
        with tile.TileContext(nc) as tc:
            _sinkhorn_kernel(tc, out, cost, src, tgt)
        nc.compile()
        _CACHE["nc"] = nc
    return _CACHE["nc"]


def kernel(cost, source_marginal, target_marginal):
    cost = np.ascontiguousarray(cost, dtype=np.float32)
    src = np.ascontiguousarray(source_marginal, dtype=np.float32)
    tgt = np.ascontiguousarray(target_marginal, dtype=np.float32)
    B = cost.shape[0]
    assert B == BPC * NCORES
    nc = _get_compiled()
    in_maps = [
        {
            "cost": cost[k * BPC:(k + 1) * BPC],
            "src": src[k * BPC:(k + 1) * BPC],
            "tgt": tgt[k * BPC:(k + 1) * BPC],
        }
        for k in range(NCORES)
    ]
    res = run_bass_kernel_spmd(nc, in_maps, list(range(NCORES))).results
    return np.concatenate([res[k]["out"] for k in range(NCORES)], axis=0)
